# revision 30
# baseline (speedup 1.0000x reference)
"""LocallyConnected2d Bass kernel for 8 Trainium2 NeuronCores.

Problem (hardcoded): x[16,32,64,64] f32, weight[64,64,32,32,3,3] f32,
bias[32,64,64] f32 -> out[16,32,64,64] f32.  stride=1, pad=1, dil=1.

Sharding: outH split across 8 cores (8 rows each).  Per core, per output
row h: 64 w-positions x 3 kernel-rows of matmuls [K<=97,M=32]x[K,N=16]
accumulated in PSUM.  K = (kernel-col j)*32 + inC c, with a 97th "ones"
row carrying the bias.

Traffic optimizations vs the bf16 baseline:
  - weights (the dominant stream) stored fp8 e3m4, scaled by 2^8 on host
    (output descaled by 2^-8 on host - exact).  Halves weight HBM bytes.
  - x is DMAed once (unreplicated, [32c, 10hh, 66wp*16b] bf16) into
    partitions 0..31; the kernel-column-shifted copies for partition
    groups 1,2 (j=1,2) are made on-chip by DVE partition-shifted copies
    (4x perf mode), cutting x HBM bytes 3x.
  - one weight DMA per output row (row 7 in two halves to shorten the
    dependent tail); out DMAs issued on the sync ring after all weight
    DMAs so weights win the DMA-engine queue.

w-positions are processed in quads: position w = q*4+g is computed by a
matmul col-tiled to column group g (tile_position=(0,32g)), so the four
LDWEIGHTS+MATMUL streams of a quad run concurrently in the PE array.
PSUM tile is [128 = 4w x 32o, 16 quads x 16b] per output row.
"""

import numpy as np
import ml_dtypes

B, C, H, W = 16, 32, 64, 64
OC = 32
KH = KW = 3
NCORES = 8
RPC = H // NCORES  # rows per core = 8
NQ = 4  # quad size (PE col groups)
WSCALE = 2.0**8  # weight scale into fp8e3 range (max 15.08 < 15.5)
WP = W + 2  # padded width positions per row

BF16 = ml_dtypes.bfloat16
F8E3 = ml_dtypes.float8_e3m4

# x tile chunking by padded row hh: chunk -> (hh0, hh1)
XCHUNKS = [(0, 3), (3, 7), (7, 10)]

_cache = {}


def _build_nc():
    import concourse.bass as bass
    import concourse.tile as tile
    from concourse import bacc, mybir

    nc = bacc.Bacc(
        "TRN2", target_bir_lowering=False, debug=False, num_devices=NCORES
    )
    f32 = mybir.dt.float32
    f16 = mybir.dt.float16
    bf16 = mybir.dt.bfloat16
    f8e3 = mybir.dt.float8e3

    # xs: [33, 10, 66*16] bf16.  Partition c<32 holds x[c, hh, wp, b]
    # (hh = local padded row 0..9, wp = padded col 0..65, b = batch);
    # partition 32 is all-ones (bias row) so it rides the same DMA.
    xs = nc.dram_tensor("xs", (33, 10, WP * B), bf16, kind="ExternalInput")
    # wt: [8, 97, 64*3*32] f8e3, scaled by 2^8; [h, j*32+c, (w*3+ik)*32+o];
    # row 96 = bias (nonzero only at ik==2 slots).
    wt = nc.dram_tensor(
        "wt", (RPC, 97, W * KH * OC), f8e3, kind="ExternalInput"
    )
    # out: [8, 128, 16*16] f16 = 2^8 * out[h, g*32+o, q*16+b] with w = q*4+g
    out = nc.dram_tensor(
        "out", (RPC, 4 * OC, (W // NQ) * B), f16, kind="ExternalOutput"
    )

    with tile.TileContext(nc) as tc:
        with (
            tc.tile_pool(name="xpool", bufs=1) as xpool,
            tc.tile_pool(name="wpool", bufs=1) as wpool,
            tc.tile_pool(name="opool", bufs=1) as opool,
            tc.tile_pool(name="psum", bufs=3, space="PSUM") as ppool,
            tc.tile_pool(name="psum7", bufs=1, space="PSUM") as ppool7,
        ):
            # Per-chunk x tiles [97, rows, WP*16].  DMA x once into
            # partitions 0..31, ones row into 96; DVE makes the j=1,2
            # column-shifted copies into partitions 32..95.  Everything
            # rides the single sync ring in a hand-ordered sequence so
            # the serialized DMA engines see x0,x1,x2,w0,ones,w1,... and
            # the HWDGE descriptor-gen (~0.63us/DMA) stays ahead of the
            # transfer stream.
            xtiles = []
            for ci, (h0, h1) in enumerate(XCHUNKS):
                r = h1 - h0
                t = xpool.tile([97, r, WP * B], bf16, tag=f"x{ci}")
                xtiles.append(t)
            # x0 rides the gpsimd SWDGE ring (shorter first-transfer
            # latency, keeps HWDGE free); x2 is issued before x1 so the
            # short chunk-0/2 transfers land first and chunk-0's copies
            # (which gate row 0) start as early as possible.
            for ci in (0, 2, 1):
                h0, h1 = XCHUNKS[ci]
                eng = nc.gpsimd if ci == 0 else nc.sync
                eng.dma_start(xtiles[ci][0:33], xs[:, h0:h1])

            def xcopies(ci):
                # j=2 first and spanning 33 partitions: its source row 32
                # (the DMAed all-ones row) lands on partition 96, giving
                # the bias ones-row for free.  j=1 then overwrites
                # partition 32 (safe: emitted after j=2's read).
                t = xtiles[ci]
                nc.vector.tensor_copy(
                    t[64:97, :, 0 : W * B], t[0:33, :, 2 * B : 2 * B + W * B]
                )
                nc.vector.tensor_copy(
                    t[32:64, :, 0 : W * B], t[0:32, :, B : B + W * B]
                )

            def xslice(hh, w, k):
                for (h0, h1), t in zip(XCHUNKS, xtiles):
                    if h0 <= hh < h1:
                        return t[0:k, hh - h0, w * B : (w + 1) * B]
                raise AssertionError

            # Weight DMAs, one tile per row, loaded in quad-range pieces:
            # rows 0..3 whole, rows 4..6 in halves, row 7 in quarters.
            # Finer pieces toward the end shorten "weight bytes not yet
            # arrived when their dependent compute remains" without
            # letting the ~0.63us/DMA HWDGE cost outrun the transfers.
            WPIECES = {h: [(0, 16)] for h in range(4)}
            WPIECES.update({h: [(0, 8), (8, 16)] for h in (4, 5, 6)})
            WPIECES[7] = [(0, 4), (4, 8), (8, 12), (12, 16)]
            wtiles = {h: [] for h in range(RPC)}  # [(q0, q1, tile), ...]

            def load_w(h):
                for pi, (q0, q1) in enumerate(WPIECES[h]):
                    c0, c1 = q0 * NQ * KH * OC, q1 * NQ * KH * OC
                    t = wpool.tile([97, c1 - c0], f8e3, tag=f"w{h}_{pi}")
                    nc.sync.dma_start(t[:], wt[h, :, c0:c1])
                    wtiles[h].append((q0 * NQ, q1 * NQ, t))

            for h in range(RPC):
                load_w(h)

            def wslice(h, w, ik, k):
                for w0, w1, t in wtiles[h]:
                    if w0 <= w < w1:
                        return t[0:k, ((w - w0) * 3 + ik) * 32 :][:, 0:32]
                raise AssertionError

            # x replication copies for chunks 0,1 ahead of all PSUM
            # copies in the DVE queue; chunk 2 (needed from row 5) is
            # emitted after row 1 so rows 0-1's PSUM copies aren't stuck
            # behind it.
            xcopies(0)
            xcopies(1)

            def mm_quads(h, pt, q0, q1, pq0):
                for q in range(q0, q1):
                    for g in range(NQ):
                        w = q * NQ + g
                        for ik in range(KH):
                            nc.tensor.matmul(
                                pt[
                                    32 * g : 32 * (g + 1),
                                    (q - pq0) * B : (q - pq0 + 1) * B,
                                ],
                                wslice(h, w, ik, 97),
                                xslice(h + ik, w, 97),
                                start=(ik == 0),
                                stop=(ik == 2),
                                tile_position=(0, 32 * g),
                            )

            outs = []  # (dram row, sbuf tile) deferred out DMAs
            NQW = W // NQ  # 16 quads per row
            for h in range(RPC):
                ot = opool.tile([4 * OC, NQW * B], f16, tag=f"o{h}")
                if h == RPC - 1:
                    # separate PSUM tile + copy per weight piece (PSUM
                    # dependencies are tile-granular: sharing one tile
                    # would serialize piece k+1's matmuls on piece k's
                    # copy); one out DMA for the row.
                    for pi, (q0, q1) in enumerate(WPIECES[h]):
                        pt = ppool7.tile(
                            [4 * OC, (q1 - q0) * B], f32, tag=f"p7{pi}"
                        )
                        mm_quads(h, pt, q0, q1, q0)
                        nc.vector.tensor_copy(
                            ot[:, q0 * B : q1 * B], pt[:]
                        )
                else:
                    pt = ppool.tile([4 * OC, NQW * B], f32)
                    mm_quads(h, pt, 0, NQW, 0)
                    nc.vector.tensor_copy(ot[:], pt[:])
                outs.append((out[h], ot))
                if h == 1:
                    xcopies(2)

            # out DMAs issued after all weight DMAs so weight transfers
            # win the DMA-engine queue.  The last rows alternate onto the
            # scalar ring: each out's SEQ is held through its copy-wait,
            # so a second ring keeps the final out's launch off the
            # serialized chain of earlier outs.
            for h, (osl, ot) in enumerate(outs):
                eng = nc.scalar if h >= 5 and h % 2 == 1 else nc.sync
                eng.dma_start(osl, ot[:])
    nc.compile()
    return nc


def _prep_inputs(x, weight, bias):
    """Host-side shard + layout prep.  Returns list of 8 per-core dicts."""
    # padded x, transposed to [c, hh, wp, b]
    xp = np.zeros((C, H + 2, W + 2, B), dtype=BF16)
    xp[:, 1 : H + 1, 1 : W + 1, :] = np.ascontiguousarray(
        x.transpose(1, 2, 3, 0)
    ).astype(BF16)

    # weight -> [h, j, c, w, ik, o], scaled into fp8e3 range
    wtr = np.ascontiguousarray(
        weight.transpose(0, 5, 3, 1, 4, 2) * np.float32(WSCALE)
    ).astype(F8E3)
    wtr = wtr.reshape(H, 96, W, KH, OC)
    btr = (bias.transpose(1, 2, 0) * np.float32(WSCALE)).astype(F8E3)  # [h,w,o]

    in_maps = []
    for i in range(NCORES):
        h0 = i * RPC
        xcore = np.ones((33, RPC + 2, WP, B), dtype=BF16)
        xcore[0:32] = xp[:, h0 : h0 + RPC + 2, :, :]

        # partition map: j*32+c for j=0,1,2; 96 = bias row (nonzero only
        # at ik==2 slots, multiplied by the ones-row in x).
        wcore = np.zeros((RPC, 97, W, KH, OC), dtype=F8E3)
        wcore[:, 0:96] = wtr[h0 : h0 + RPC]
        wcore[:, 96, :, 2, :] = btr[h0 : h0 + RPC]

        in_maps.append(
            {
                "xs": np.ascontiguousarray(xcore.reshape(33, RPC + 2, WP * B)),
                "wt": np.ascontiguousarray(
                    wcore.reshape(RPC, 97, W * KH * OC)
                ),
            }
        )
    return in_maps


def _run(in_maps, trace=False, tmpdir=None):
    from concourse.bass_utils import run_bass_kernel_spmd

    if "nc" not in _cache:
        _cache["nc"] = _build_nc()
    return run_bass_kernel_spmd(
        _cache["nc"], in_maps, list(range(NCORES)), trace=trace, tmpdir=tmpdir
    )


def _assemble(results):
    out = np.empty((B, OC, H, W), dtype=np.float32)
    inv = np.float32(1.0 / WSCALE)
    for i in range(NCORES):
        # res: [h, g*32+o, q*16+b], w = q*4+g
        res = (
            results[i]["out"].astype(np.float32).reshape(RPC, NQ, OC, W // NQ, B)
            * inv
        )
        # -> out[b, o, h, q*4+g]
        out[:, :, i * RPC : (i + 1) * RPC, :] = res.transpose(
            4, 2, 0, 3, 1
        ).reshape(B, OC, RPC, W)
    return out


def kernel(x, weight, bias):
    x = np.asarray(x)
    weight = np.asarray(weight)
    bias = np.asarray(bias)
    in_maps = _prep_inputs(x, weight, bias)
    results = _run(in_maps).results
    return _assemble(results)


# revision 31
# speedup vs baseline: 1.0275x; 1.0275x over previous
"""LocallyConnected2d Bass kernel for 8 Trainium2 NeuronCores.

Problem (hardcoded): x[16,32,64,64] f32, weight[64,64,32,32,3,3] f32,
bias[32,64,64] f32 -> out[16,32,64,64] f32.  stride=1, pad=1, dil=1.

Sharding: outH split across 8 cores (8 rows each).  Per core, per output
row h: 64 w-positions x 3 kernel-rows of matmuls [K<=97,M=32]x[K,N=16]
accumulated in PSUM.  K = (kernel-col j)*32 + inC c, with a 97th "ones"
row carrying the bias.

Traffic optimizations vs the bf16 baseline:
  - weights (the dominant stream) stored fp8 e3m4, scaled by 2^8 on host
    (output descaled by 2^-8 on host - exact).  Halves weight HBM bytes.
  - x is DMAed once (unreplicated, [32c, 10hh, 66wp*16b] bf16) into
    partitions 0..31; the kernel-column-shifted copies for partition
    groups 1,2 (j=1,2) are made on-chip by DVE partition-shifted copies
    (4x perf mode), cutting x HBM bytes 3x.
  - one weight DMA per output row (row 7 in two halves to shorten the
    dependent tail); out DMAs issued on the sync ring after all weight
    DMAs so weights win the DMA-engine queue.

w-positions are processed in quads: position w = q*4+g is computed by a
matmul col-tiled to column group g (tile_position=(0,32g)), so the four
LDWEIGHTS+MATMUL streams of a quad run concurrently in the PE array.
PSUM tile is [128 = 4w x 32o, 16 quads x 16b] per output row.
"""

import numpy as np
import ml_dtypes

B, C, H, W = 16, 32, 64, 64
OC = 32
KH = KW = 3
NCORES = 8
RPC = H // NCORES  # rows per core = 8
NQ = 4  # quad size (PE col groups)
WSCALE = 2.0**8  # weight scale into fp8e3 range (max 15.08 < 15.5)
WP = W + 2  # padded width positions per row

BF16 = ml_dtypes.bfloat16
F8E3 = ml_dtypes.float8_e3m4

# x tile chunking by padded row hh: chunk -> (hh0, hh1)
XCHUNKS = [(0, 3), (3, 7), (7, 10)]

_cache = {}


def _build_nc():
    import concourse.bass as bass
    import concourse.tile as tile
    from concourse import bacc, mybir

    nc = bacc.Bacc(
        "TRN2", target_bir_lowering=False, debug=False, num_devices=NCORES
    )
    f32 = mybir.dt.float32
    f16 = mybir.dt.float16
    bf16 = mybir.dt.bfloat16
    f8e3 = mybir.dt.float8e3

    # xs: [33, 10, 66*16] bf16.  Partition c<32 holds x[c, hh, wp, b]
    # (hh = local padded row 0..9, wp = padded col 0..65, b = batch);
    # partition 32 is all-ones (bias row) so it rides the same DMA.
    xs = nc.dram_tensor("xs", (33, 10, WP * B), bf16, kind="ExternalInput")
    # wt: [8, 97, 64*3*32] f8e3, scaled by 2^8; [h, j*32+c, (w*3+ik)*32+o];
    # row 96 = bias (nonzero only at ik==2 slots).
    wt = nc.dram_tensor(
        "wt", (RPC, 97, W * KH * OC), f8e3, kind="ExternalInput"
    )
    # out: [8, 128, 16*16] f16 = 2^8 * out[h, g*32+o, q*16+b] with w = q*4+g
    out = nc.dram_tensor(
        "out", (RPC, 4 * OC, (W // NQ) * B), f16, kind="ExternalOutput"
    )

    with tile.TileContext(nc) as tc:
        with (
            tc.tile_pool(name="xpool", bufs=1) as xpool,
            tc.tile_pool(name="wpool", bufs=1) as wpool,
            tc.tile_pool(name="opool", bufs=1) as opool,
            tc.tile_pool(name="psum", bufs=3, space="PSUM") as ppool,
            tc.tile_pool(name="psum7", bufs=1, space="PSUM") as ppool7,
        ):
            # Per-chunk x tiles [97, rows, WP*16].  DMA x once into
            # partitions 0..31, ones row into 96; DVE makes the j=1,2
            # column-shifted copies into partitions 32..95.  Everything
            # rides the single sync ring in a hand-ordered sequence so
            # the serialized DMA engines see x0,x1,x2,w0,ones,w1,... and
            # the HWDGE descriptor-gen (~0.63us/DMA) stays ahead of the
            # transfer stream.
            xtiles = []
            for ci, (h0, h1) in enumerate(XCHUNKS):
                r = h1 - h0
                t = xpool.tile([97, r, WP * B], bf16, tag=f"x{ci}")
                xtiles.append(t)
            # x0 rides the gpsimd SWDGE ring (shorter first-transfer
            # latency, keeps HWDGE free); x2 is issued before x1 so the
            # short chunk-0/2 transfers land first and chunk-0's copies
            # (which gate row 0) start as early as possible.
            for ci in (0, 2, 1):
                h0, h1 = XCHUNKS[ci]
                eng = nc.gpsimd if ci == 0 else nc.sync
                eng.dma_start(xtiles[ci][0:33], xs[:, h0:h1])

            def xcopies(ci):
                # j=2 first and spanning 33 partitions: its source row 32
                # (the DMAed all-ones row) lands on partition 96, giving
                # the bias ones-row for free.  j=1 then overwrites
                # partition 32 (safe: emitted after j=2's read).
                t = xtiles[ci]
                nc.vector.tensor_copy(
                    t[64:97, :, 0 : W * B], t[0:33, :, 2 * B : 2 * B + W * B]
                )
                nc.vector.tensor_copy(
                    t[32:64, :, 0 : W * B], t[0:32, :, B : B + W * B]
                )

            def xslice(hh, w, k):
                for (h0, h1), t in zip(XCHUNKS, xtiles):
                    if h0 <= hh < h1:
                        return t[0:k, hh - h0, w * B : (w + 1) * B]
                raise AssertionError

            # Weight DMAs, one tile per row, loaded in quad-range pieces:
            # rows 0..3 whole, rows 4..6 in halves, row 7 in quarters.
            # Finer pieces toward the end shorten "weight bytes not yet
            # arrived when their dependent compute remains" without
            # letting the ~0.63us/DMA HWDGE cost outrun the transfers.
            WPIECES = {h: [(0, 16)] for h in range(4)}
            WPIECES.update({h: [(0, 8), (8, 16)] for h in (4, 5, 6)})
            WPIECES[7] = [(0, 4), (4, 8), (8, 12), (12, 16)]
            wtiles = {h: [] for h in range(RPC)}  # [(q0, q1, tile), ...]

            def load_w(h):
                for pi, (q0, q1) in enumerate(WPIECES[h]):
                    c0, c1 = q0 * NQ * KH * OC, q1 * NQ * KH * OC
                    t = wpool.tile([97, c1 - c0], f8e3, tag=f"w{h}_{pi}")
                    nc.sync.dma_start(t[:], wt[h, :, c0:c1])
                    wtiles[h].append((q0 * NQ, q1 * NQ, t))

            for h in range(RPC):
                load_w(h)

            def wslice(h, w, ik, k):
                for w0, w1, t in wtiles[h]:
                    if w0 <= w < w1:
                        return t[0:k, ((w - w0) * 3 + ik) * 32 :][:, 0:32]
                raise AssertionError

            # x replication copies for chunks 0,1 ahead of all PSUM
            # copies in the DVE queue; chunk 2 (needed from row 5) is
            # emitted after row 1 so rows 0-1's PSUM copies aren't stuck
            # behind it.
            xcopies(0)
            xcopies(1)

            def mm_quads(h, pt, q0, q1, pq0):
                for q in range(q0, q1):
                    for g in range(NQ):
                        w = q * NQ + g
                        for ik in range(KH):
                            nc.tensor.matmul(
                                pt[
                                    32 * g : 32 * (g + 1),
                                    (q - pq0) * B : (q - pq0 + 1) * B,
                                ],
                                wslice(h, w, ik, 97),
                                xslice(h + ik, w, 97),
                                start=(ik == 0),
                                stop=(ik == 2),
                                tile_position=(0, 32 * g),
                            )

            outs = []  # (dram row, sbuf tile) deferred out DMAs
            NQW = W // NQ  # 16 quads per row
            for h in range(RPC):
                ot = opool.tile([4 * OC, NQW * B], f16, tag=f"o{h}")
                if h == RPC - 1:
                    # separate PSUM tile + copy per weight piece (PSUM
                    # dependencies are tile-granular: sharing one tile
                    # would serialize piece k+1's matmuls on piece k's
                    # copy); one out DMA for the row.
                    for pi, (q0, q1) in enumerate(WPIECES[h]):
                        pt = ppool7.tile(
                            [4 * OC, (q1 - q0) * B], f32, tag=f"p7{pi}"
                        )
                        mm_quads(h, pt, q0, q1, q0)
                        nc.vector.tensor_copy(
                            ot[:, q0 * B : q1 * B], pt[:]
                        )
                else:
                    pt = ppool.tile([4 * OC, NQW * B], f32)
                    mm_quads(h, pt, 0, NQW, 0)
                    nc.vector.tensor_copy(ot[:], pt[:])
                outs.append((out[h], ot))
                if h == 1:
                    xcopies(2)

            # out DMAs issued after all weight DMAs so weight transfers
            # win the DMA-engine queue.  The last rows alternate onto the
            # scalar ring: each out's SEQ is held through its copy-wait,
            # so a second ring keeps the final out's launch off the
            # serialized chain of earlier outs.
            for h, (osl, ot) in enumerate(outs):
                nc.sync.dma_start(osl, ot[:])
    nc.compile()
    return nc


def _prep_inputs(x, weight, bias):
    """Host-side shard + layout prep.  Returns list of 8 per-core dicts."""
    # padded x, transposed to [c, hh, wp, b]
    xp = np.zeros((C, H + 2, W + 2, B), dtype=BF16)
    xp[:, 1 : H + 1, 1 : W + 1, :] = np.ascontiguousarray(
        x.transpose(1, 2, 3, 0)
    ).astype(BF16)

    # weight -> [h, j, c, w, ik, o], scaled into fp8e3 range
    wtr = np.ascontiguousarray(
        weight.transpose(0, 5, 3, 1, 4, 2) * np.float32(WSCALE)
    ).astype(F8E3)
    wtr = wtr.reshape(H, 96, W, KH, OC)
    btr = (bias.transpose(1, 2, 0) * np.float32(WSCALE)).astype(F8E3)  # [h,w,o]

    in_maps = []
    for i in range(NCORES):
        h0 = i * RPC
        xcore = np.ones((33, RPC + 2, WP, B), dtype=BF16)
        xcore[0:32] = xp[:, h0 : h0 + RPC + 2, :, :]

        # partition map: j*32+c for j=0,1,2; 96 = bias row (nonzero only
        # at ik==2 slots, multiplied by the ones-row in x).
        wcore = np.zeros((RPC, 97, W, KH, OC), dtype=F8E3)
        wcore[:, 0:96] = wtr[h0 : h0 + RPC]
        wcore[:, 96, :, 2, :] = btr[h0 : h0 + RPC]

        in_maps.append(
            {
                "xs": np.ascontiguousarray(xcore.reshape(33, RPC + 2, WP * B)),
                "wt": np.ascontiguousarray(
                    wcore.reshape(RPC, 97, W * KH * OC)
                ),
            }
        )
    return in_maps


def _run(in_maps, trace=False, tmpdir=None):
    from concourse.bass_utils import run_bass_kernel_spmd

    if "nc" not in _cache:
        _cache["nc"] = _build_nc()
    return run_bass_kernel_spmd(
        _cache["nc"], in_maps, list(range(NCORES)), trace=trace, tmpdir=tmpdir
    )


def _assemble(results):
    out = np.empty((B, OC, H, W), dtype=np.float32)
    inv = np.float32(1.0 / WSCALE)
    for i in range(NCORES):
        # res: [h, g*32+o, q*16+b], w = q*4+g
        res = (
            results[i]["out"].astype(np.float32).reshape(RPC, NQ, OC, W // NQ, B)
            * inv
        )
        # -> out[b, o, h, q*4+g]
        out[:, :, i * RPC : (i + 1) * RPC, :] = res.transpose(
            4, 2, 0, 3, 1
        ).reshape(B, OC, RPC, W)
    return out


def kernel(x, weight, bias):
    x = np.asarray(x)
    weight = np.asarray(weight)
    bias = np.asarray(bias)
    in_maps = _prep_inputs(x, weight, bias)
    results = _run(in_maps).results
    return _assemble(results)


# revision 43
# speedup vs baseline: 1.0403x; 1.0124x over previous
"""LocallyConnected2d Bass kernel for 8 Trainium2 NeuronCores.

Problem (hardcoded): x[16,32,64,64] f32, weight[64,64,32,32,3,3] f32,
bias[32,64,64] f32 -> out[16,32,64,64] f32.  stride=1, pad=1, dil=1.

Sharding: outH split across 8 cores (8 rows each).  Per core, per output
row h: 64 w-positions x 3 kernel-rows of matmuls [K=96,M=32]x[K,N=16]
accumulated in PSUM, K = (kernel-col j)*32 + inC c.  The kernel is HBM
-bandwidth bound on the per-position weights, so the design minimizes
bytes and keeps the (serialized) DMA-engine stream dense:
  - weights (the dominant stream) stored fp8 e3m4, scaled by 2^8 on host
    (output descaled by 2^-8 on host - exact).  Halves weight HBM bytes;
    x stays bf16 (all-fp8 would breach the accuracy budget).
  - x is DMAed once (unreplicated, [32c, hh, 66wp*16b] bf16) into
    partitions 0..31; the kernel-column-shifted copies for partition
    groups j=1,2 are made on-chip by DVE partition-shifted copies
    (4x perf mode), cutting x HBM bytes 3x.
  - bias is added during the PSUM->SBUF copy (DVE tensor_add with a
    stride-0 broadcast AP over batch), so no bias row rides the K dim.
  - weight DMAs are row-granular early and finer toward the end (halves
    for rows 4-6, thirds for row 7) so little dependent compute remains
    after the last weight byte lands, without letting the ~0.63us/DMA
    HWDGE descriptor-gen cost outrun the transfer stream.
  - out DMAs issue after all weight DMAs so weights win the DMA queue.

w-positions are processed in quads: position w = q*4+g is computed by a
matmul col-tiled to column group g (tile_position=(0,32g)), so the four
LDWEIGHTS+MATMUL streams of a quad run concurrently in the PE array.
PSUM tile is [128 = 4g x 32o, 16 quads x 16b] per output row.
"""

import numpy as np
import ml_dtypes

B, C, H, W = 16, 32, 64, 64
OC = 32
KH = KW = 3
NCORES = 8
RPC = H // NCORES  # rows per core = 8
NQ = 4  # quad size (PE col groups)
WSCALE = 2.0**8  # weight scale into fp8e3 range (max 15.08 < 15.5)
WP = W + 2  # padded width positions per row

BF16 = ml_dtypes.bfloat16
F8E3 = ml_dtypes.float8_e3m4

# x tile chunking by padded row hh: chunk -> (hh0, hh1)
XCHUNKS = [(0, 3), (3, 7), (7, 10)]

_cache = {}


def _build_nc():
    import concourse.bass as bass
    import concourse.tile as tile
    from concourse import bacc, mybir

    nc = bacc.Bacc(
        "TRN2", target_bir_lowering=False, debug=False, num_devices=NCORES
    )
    f32 = mybir.dt.float32
    f16 = mybir.dt.float16
    bf16 = mybir.dt.bfloat16
    f8e3 = mybir.dt.float8e3

    # xs: [32, 10, 66*16] bf16.  Partition c holds x[c, hh, wp, b]
    # (hh = local padded row 0..9, wp = padded col 0..65, b = batch).
    xs = nc.dram_tensor("xs", (32, 10, WP * B), bf16, kind="ExternalInput")
    # wt: [8, 96, 64*3*32] f8e3, scaled by 2^8; [h, j*32+c, (w*3+ik)*32+o].
    wt = nc.dram_tensor(
        "wt", (RPC, 96, W * KH * OC), f8e3, kind="ExternalInput"
    )
    # bsc: [128, 8*16] f32 = 2^8 * bias[o, h, w] at [g*32+o, h*16+q],
    # w = q*4+g; added during the PSUM->SBUF copy with a b-broadcast AP.
    bsc = nc.dram_tensor(
        "bsc", (4 * OC, RPC * (W // NQ)), f32, kind="ExternalInput"
    )
    # out: [8, 128, 16*16] f16 = 2^8 * out[h, g*32+o, q*16+b] with w = q*4+g
    out = nc.dram_tensor(
        "out", (RPC, 4 * OC, (W // NQ) * B), f16, kind="ExternalOutput"
    )

    with tile.TileContext(nc) as tc:
        with (
            tc.tile_pool(name="xpool", bufs=1) as xpool,
            tc.tile_pool(name="wpool", bufs=1) as wpool,
            tc.tile_pool(name="opool", bufs=1) as opool,
            tc.tile_pool(name="psum", bufs=3, space="PSUM") as ppool,
            tc.tile_pool(name="psum7", bufs=1, space="PSUM") as ppool7,
        ):
            # Per-chunk x tiles [96, rows, WP*16].  DMA x once into
            # partitions 0..31; DVE makes the j=1,2 column-shifted
            # copies into partitions 32..95.
            xtiles = []
            for ci, (h0, h1) in enumerate(XCHUNKS):
                r = h1 - h0
                t = xpool.tile([96, r, WP * B], bf16, tag=f"x{ci}")
                xtiles.append(t)
            bt = xpool.tile([4 * OC, RPC * (W // NQ)], f32, tag="bias")
            nc.scalar.dma_start(bt[:], bsc[:, :])
            # x0 rides the gpsimd SWDGE ring: shorter first-transfer
            # latency than HWDGE and it keeps the HWDGE queue free for
            # the weight stream.
            def load_x(ci):
                h0, h1 = XCHUNKS[ci]
                eng = nc.gpsimd if ci == 0 else nc.sync
                eng.dma_start(xtiles[ci][0:32], xs[:, h0:h1])

            load_x(0)

            def xcopies(ci):
                t = xtiles[ci]
                for j in (1, 2):
                    nc.vector.tensor_copy(
                        t[32 * j : 32 * (j + 1), :, 0 : W * B],
                        t[0:32, :, j * B : j * B + W * B],
                    )

            def xslice(hh, w, k):
                for (h0, h1), t in zip(XCHUNKS, xtiles):
                    if h0 <= hh < h1:
                        return t[0:k, hh - h0, w * B : (w + 1) * B]
                raise AssertionError

            # Weight DMAs, one tile per quad-range piece: rows 0..3
            # whole, rows 4..6 in halves, row 7 in thirds.  Finer pieces
            # toward the end shorten "weight bytes not yet arrived while
            # their dependent compute remains" without letting the
            # ~0.63us/DMA HWDGE cost outrun the transfers.
            WPIECES = {h: [(0, 16)] for h in range(4)}
            WPIECES.update({h: [(0, 8), (8, 16)] for h in (4, 5, 6)})
            WPIECES[7] = [(0, 7), (7, 12), (12, 16)]
            wtiles = {h: [] for h in range(RPC)}  # [(q0, q1, tile), ...]

            def load_w(h):
                for pi, (q0, q1) in enumerate(WPIECES[h]):
                    c0, c1 = q0 * NQ * KH * OC, q1 * NQ * KH * OC
                    t = wpool.tile([96, c1 - c0], f8e3, tag=f"w{h}_{pi}")
                    nc.sync.dma_start(t[:], wt[h, :, c0:c1])
                    wtiles[h].append((q0 * NQ, q1 * NQ, t))

            load_x(1)
            for h in range(RPC):
                load_w(h)
                if h == 1:
                    load_x(2)

            def wslice(h, w, ik, k):
                for w0, w1, t in wtiles[h]:
                    if w0 <= w < w1:
                        return t[0:k, ((w - w0) * 3 + ik) * 32 :][:, 0:32]
                raise AssertionError

            # x replication copies for chunks 0,1 ahead of all PSUM
            # copies in the DVE queue; chunk 2 (needed from row 5) is
            # emitted after row 1 so rows 0-1's PSUM copies aren't stuck
            # behind it.
            xcopies(0)
            xcopies(1)

            def bias_bcast(h, q0, q1):
                # [128, q1-q0] bias slice with a stride-0 batch dim so it
                # broadcasts across the 16 batch columns of each quad.
                a = bt[0 : 4 * OC, h * (W // NQ) + q0 : h * (W // NQ) + q1]
                return bass.AP(a.tensor, a.offset, list(a.ap) + [[0, B]])

            def mm_quads(h, pt, q0, q1, pq0):
                for q in range(q0, q1):
                    for g in range(NQ):
                        w = q * NQ + g
                        for ik in range(KH):
                            nc.tensor.matmul(
                                pt[
                                    32 * g : 32 * (g + 1),
                                    (q - pq0) * B : (q - pq0 + 1) * B,
                                ],
                                wslice(h, w, ik, 96),
                                xslice(h + ik, w, 96),
                                start=(ik == 0),
                                stop=(ik == 2),
                                tile_position=(0, 32 * g),
                            )

            outs = []  # (dram row, sbuf tile) deferred out DMAs
            NQW = W // NQ  # 16 quads per row
            for h in range(RPC):
                ot = opool.tile([4 * OC, NQW * B], f16, tag=f"o{h}")
                if h == RPC - 1:
                    # separate PSUM tile + copy per weight piece (PSUM
                    # dependencies are tile-granular: sharing one tile
                    # would serialize piece k+1's matmuls on piece k's
                    # copy); one out DMA for the row.
                    for pi, (q0, q1) in enumerate(WPIECES[h]):
                        pt = ppool7.tile(
                            [4 * OC, (q1 - q0) * B], f32, tag=f"p7{pi}"
                        )
                        mm_quads(h, pt, q0, q1, q0)
                        nc.vector.tensor_add(
                            ot[:, q0 * B : q1 * B], pt[:],
                            bias_bcast(h, q0, q1),
                        )
                else:
                    pt = ppool.tile([4 * OC, NQW * B], f32)
                    mm_quads(h, pt, 0, NQW, 0)
                    nc.vector.tensor_add(ot[:], pt[:], bias_bcast(h, 0, NQW))
                outs.append((out[h], ot))
                if h == 1:
                    xcopies(2)

            # out DMAs issued after all weight DMAs so weight transfers
            # win the DMA-engine queue.
            for osl, ot in outs:
                nc.sync.dma_start(osl, ot[:])
    nc.compile()
    return nc


def _prep_inputs(x, weight, bias):
    """Host-side shard + layout prep.  Returns list of 8 per-core dicts."""
    # padded x, transposed to [c, hh, wp, b]
    xp = np.zeros((C, H + 2, W + 2, B), dtype=BF16)
    xp[:, 1 : H + 1, 1 : W + 1, :] = np.ascontiguousarray(
        x.transpose(1, 2, 3, 0)
    ).astype(BF16)

    # weight -> [h, j, c, w, ik, o], scaled into fp8e3 range
    wtr = np.ascontiguousarray(
        weight.transpose(0, 5, 3, 1, 4, 2) * np.float32(WSCALE)
    ).astype(F8E3)
    wtr = wtr.reshape(H, 96, W, KH, OC)
    btr = bias.astype(np.float32) * np.float32(WSCALE)  # [o, h, w] exact

    in_maps = []
    for i in range(NCORES):
        h0 = i * RPC
        xcore = xp[:, h0 : h0 + RPC + 2, :, :]
        wcore = wtr[h0 : h0 + RPC]
        # bsc[g*32+o, h*16+q] = 2^8 * bias[o, h0+h, q*4+g]
        bcore = btr[:, h0 : h0 + RPC]  # [o, h, w]
        bcore = bcore.reshape(OC, RPC, W // NQ, NQ)
        bcore = np.ascontiguousarray(
            bcore.transpose(3, 0, 1, 2)  # [g, o, h, q]
        ).reshape(4 * OC, RPC * (W // NQ))

        in_maps.append(
            {
                "xs": np.ascontiguousarray(
                    xcore.reshape(32, RPC + 2, WP * B)
                ),
                "wt": np.ascontiguousarray(
                    wcore.reshape(RPC, 96, W * KH * OC)
                ),
                "bsc": bcore,
            }
        )
    return in_maps


def _run(in_maps, trace=False, tmpdir=None):
    from concourse.bass_utils import run_bass_kernel_spmd

    if "nc" not in _cache:
        _cache["nc"] = _build_nc()
    return run_bass_kernel_spmd(
        _cache["nc"], in_maps, list(range(NCORES)), trace=trace, tmpdir=tmpdir
    )


def _assemble(results):
    out = np.empty((B, OC, H, W), dtype=np.float32)
    inv = np.float32(1.0 / WSCALE)
    for i in range(NCORES):
        # res: [h, g*32+o, q*16+b], w = q*4+g
        res = (
            results[i]["out"].astype(np.float32).reshape(RPC, NQ, OC, W // NQ, B)
            * inv
        )
        # -> out[b, o, h, q*4+g]
        out[:, :, i * RPC : (i + 1) * RPC, :] = res.transpose(
            4, 2, 0, 3, 1
        ).reshape(B, OC, RPC, W)
    return out


def kernel(x, weight, bias):
    x = np.asarray(x)
    weight = np.asarray(weight)
    bias = np.asarray(bias)
    in_maps = _prep_inputs(x, weight, bias)
    results = _run(in_maps).results
    return _assemble(results)


# revision 50
# speedup vs baseline: 1.0473x; 1.0068x over previous
"""LocallyConnected2d Bass kernel for 8 Trainium2 NeuronCores.

Problem (hardcoded): x[16,32,64,64] f32, weight[64,64,32,32,3,3] f32,
bias[32,64,64] f32 -> out[16,32,64,64] f32.  stride=1, pad=1, dil=1.

Sharding: outH split across 8 cores (8 rows each).  Per core, per output
row h: 64 w-positions x 3 kernel-rows of matmuls [K=96,M=32]x[K,N=16]
accumulated in PSUM, K = (kernel-col j)*32 + inC c.  The kernel is HBM
-bandwidth bound on the per-position weights, so the design minimizes
bytes and keeps the (serialized) DMA-engine stream dense:
  - weights (the dominant stream) stored fp8 e3m4, scaled by 2^8 on host
    (output descaled by 2^-8 on host - exact).  Halves weight HBM bytes;
    x stays bf16 (all-fp8 would breach the accuracy budget).
  - x is DMAed once (unreplicated, [32c, hh, 66wp*16b] bf16) into
    partitions 0..31; the kernel-column-shifted copies for partition
    groups j=1,2 are made on-chip by DVE partition-shifted copies
    (4x perf mode), cutting x HBM bytes 3x.
  - bias is added during the PSUM->SBUF copy (DVE tensor_add with a
    stride-0 broadcast AP over batch), so no bias row rides the K dim.
  - weight DMAs are row-granular early and finer toward the end (halves
    for rows 4-6, thirds for row 7) so little dependent compute remains
    after the last weight byte lands, without letting the ~0.63us/DMA
    HWDGE descriptor-gen cost outrun the transfer stream.
  - out DMAs issue after all weight DMAs so weights win the DMA queue.

w-positions are processed in quads: position w = q*4+g is computed by a
matmul col-tiled to column group g (tile_position=(0,32g)), so the four
LDWEIGHTS+MATMUL streams of a quad run concurrently in the PE array.
PSUM tile is [128 = 4g x 32o, 16 quads x 16b] per output row.
"""

import numpy as np
import ml_dtypes

B, C, H, W = 16, 32, 64, 64
OC = 32
KH = KW = 3
NCORES = 8
RPC = H // NCORES  # rows per core = 8
NQ = 4  # quad size (PE col groups)
WSCALE = 2.0**8  # weight scale into fp8e3 range (max 15.08 < 15.5)
WP = W + 2  # padded width positions per row

BF16 = ml_dtypes.bfloat16
F8E3 = ml_dtypes.float8_e3m4

# x tile chunking by padded row hh: chunk -> (hh0, hh1)
XCHUNKS = [(0, 3), (3, 7), (7, 10)]

_cache = {}


def _build_nc():
    import concourse.bass as bass
    import concourse.tile as tile
    from concourse import bacc, mybir

    nc = bacc.Bacc(
        "TRN2", target_bir_lowering=False, debug=False, num_devices=NCORES
    )
    f32 = mybir.dt.float32
    f16 = mybir.dt.float16
    bf16 = mybir.dt.bfloat16
    f8e3 = mybir.dt.float8e3

    # xs: [32, 10, 64*16] bf16.  Partition c holds x[c, hh, wp, b] for
    # the 64 non-pad columns wp=1..64 (hh = local padded row 0..9,
    # b = batch); the zero pad columns wp=0,65 are memset on-chip.
    xs = nc.dram_tensor("xs", (32, 10, W * B), bf16, kind="ExternalInput")
    # wt: [8, 96, 64*3*32] f8e3, scaled by 2^8; [h, j*32+c, (w*3+ik)*32+o].
    wt = nc.dram_tensor(
        "wt", (RPC, 96, W * KH * OC), f8e3, kind="ExternalInput"
    )
    # bsc: [128, 8*16] f8e3 = 2^8 * bias[o, h, w] at [g*32+o, h*16+q],
    # w = q*4+g; added during the PSUM->SBUF copy with a b-broadcast AP.
    # e3m4 quantization of the bias contributes ~0.1% output error.
    bsc = nc.dram_tensor(
        "bsc", (4 * OC, RPC * (W // NQ)), f8e3, kind="ExternalInput"
    )
    # out: [8, 128, 16*16] f16 = 2^8 * out[h, g*32+o, q*16+b] with w = q*4+g
    out = nc.dram_tensor(
        "out", (RPC, 4 * OC, (W // NQ) * B), f16, kind="ExternalOutput"
    )

    with tile.TileContext(nc) as tc:
        with (
            tc.tile_pool(name="xpool", bufs=1) as xpool,
            tc.tile_pool(name="wpool", bufs=1) as wpool,
            tc.tile_pool(name="opool", bufs=1) as opool,
            tc.tile_pool(name="psum", bufs=3, space="PSUM") as ppool,
            tc.tile_pool(name="psum7", bufs=1, space="PSUM") as ppool7,
        ):
            # Per-chunk x tiles [96, rows, WP*16].  DMA x once into
            # partitions 0..31; DVE makes the j=1,2 column-shifted
            # copies into partitions 32..95.
            xtiles = []
            for ci, (h0, h1) in enumerate(XCHUNKS):
                r = h1 - h0
                t = xpool.tile([96, r, WP * B], bf16, tag=f"x{ci}")
                xtiles.append(t)
            bt = xpool.tile([4 * OC, RPC * (W // NQ)], f8e3, tag="bias")
            # x0 rides the gpsimd SWDGE ring: shorter first-transfer
            # latency than HWDGE and it keeps the HWDGE queue free for
            # the weight stream.
            def load_x(ci):
                h0, h1 = XCHUNKS[ci]
                eng = nc.gpsimd if ci == 0 else nc.sync
                eng.dma_start(
                    xtiles[ci][0:32, :, B : B + W * B], xs[:, h0:h1]
                )
                nc.vector.memset(xtiles[ci][0:32, :, 0:B], 0.0)
                nc.vector.memset(
                    xtiles[ci][0:32, :, B + W * B : WP * B], 0.0
                )

            load_x(0)

            def xcopies(ci):
                t = xtiles[ci]
                for j in (1, 2):
                    nc.vector.tensor_copy(
                        t[32 * j : 32 * (j + 1), :, 0 : W * B],
                        t[0:32, :, j * B : j * B + W * B],
                    )

            def xslice(hh, w, k):
                for (h0, h1), t in zip(XCHUNKS, xtiles):
                    if h0 <= hh < h1:
                        return t[0:k, hh - h0, w * B : (w + 1) * B]
                raise AssertionError

            # Weight DMAs, one tile per quad-range piece: rows 0..3
            # whole, rows 4..6 in halves, row 7 in thirds.  Finer pieces
            # toward the end shorten "weight bytes not yet arrived while
            # their dependent compute remains" without letting the
            # ~0.63us/DMA HWDGE cost outrun the transfers.
            WPIECES = {h: [(0, 16)] for h in range(4)}
            WPIECES.update({h: [(0, 8), (8, 16)] for h in (4, 5, 6)})
            WPIECES[7] = [(0, 7), (7, 12), (12, 16)]
            wtiles = {h: [] for h in range(RPC)}  # [(q0, q1, tile), ...]

            def load_w(h):
                for pi, (q0, q1) in enumerate(WPIECES[h]):
                    c0, c1 = q0 * NQ * KH * OC, q1 * NQ * KH * OC
                    t = wpool.tile([96, c1 - c0], f8e3, tag=f"w{h}_{pi}")
                    nc.sync.dma_start(t[:], wt[h, :, c0:c1])
                    wtiles[h].append((q0 * NQ, q1 * NQ, t))

            load_x(1)
            for h in range(RPC):
                load_w(h)
                if h == 1:
                    load_x(2)
                    nc.sync.dma_start(bt[:], bsc[:, :])

            def wslice(h, w, ik, k):
                for w0, w1, t in wtiles[h]:
                    if w0 <= w < w1:
                        return t[0:k, ((w - w0) * 3 + ik) * 32 :][:, 0:32]
                raise AssertionError

            # x replication copies for chunks 0,1 ahead of all PSUM
            # copies in the DVE queue; chunk 2 (needed from row 5) is
            # emitted after row 1 so rows 0-1's PSUM copies aren't stuck
            # behind it.
            xcopies(0)
            xcopies(1)

            def bias_bcast(h, q0, q1):
                # [128, q1-q0] bias slice with a stride-0 batch dim so it
                # broadcasts across the 16 batch columns of each quad.
                a = bt[0 : 4 * OC, h * (W // NQ) + q0 : h * (W // NQ) + q1]
                return bass.AP(a.tensor, a.offset, list(a.ap) + [[0, B]])

            def mm_quads(h, pt, q0, q1, pq0):
                for q in range(q0, q1):
                    for g in range(NQ):
                        w = q * NQ + g
                        for ik in range(KH):
                            nc.tensor.matmul(
                                pt[
                                    32 * g : 32 * (g + 1),
                                    (q - pq0) * B : (q - pq0 + 1) * B,
                                ],
                                wslice(h, w, ik, 96),
                                xslice(h + ik, w, 96),
                                start=(ik == 0),
                                stop=(ik == 2),
                                tile_position=(0, 32 * g),
                            )

            outs = []  # (dram row, sbuf tile) deferred out DMAs
            NQW = W // NQ  # 16 quads per row
            for h in range(RPC):
                ot = opool.tile([4 * OC, NQW * B], f16, tag=f"o{h}")
                if h == RPC - 1:
                    # separate PSUM tile + copy per weight piece (PSUM
                    # dependencies are tile-granular: sharing one tile
                    # would serialize piece k+1's matmuls on piece k's
                    # copy); one out DMA for the row.
                    for pi, (q0, q1) in enumerate(WPIECES[h]):
                        pt = ppool7.tile(
                            [4 * OC, (q1 - q0) * B], f32, tag=f"p7{pi}"
                        )
                        mm_quads(h, pt, q0, q1, q0)
                        nc.vector.tensor_add(
                            ot[:, q0 * B : q1 * B], pt[:],
                            bias_bcast(h, q0, q1),
                        )
                else:
                    pt = ppool.tile([4 * OC, NQW * B], f32)
                    mm_quads(h, pt, 0, NQW, 0)
                    nc.vector.tensor_add(ot[:], pt[:], bias_bcast(h, 0, NQW))
                outs.append((out[h], ot))
                if h == 1:
                    xcopies(2)

            # out DMAs issued after all weight DMAs so weight transfers
            # win the DMA-engine queue.
            for osl, ot in outs:
                nc.sync.dma_start(osl, ot[:])
    nc.compile()
    return nc


def _prep_inputs(x, weight, bias):
    """Host-side shard + layout prep.  Returns list of 8 per-core dicts."""
    # padded x, transposed to [c, hh, wp, b]
    xp = np.zeros((C, H + 2, W + 2, B), dtype=BF16)
    xp[:, 1 : H + 1, 1 : W + 1, :] = np.ascontiguousarray(
        x.transpose(1, 2, 3, 0)
    ).astype(BF16)

    # weight -> [h, j, c, w, ik, o], scaled into fp8e3 range
    wtr = np.ascontiguousarray(
        weight.transpose(0, 5, 3, 1, 4, 2) * np.float32(WSCALE)
    ).astype(F8E3)
    wtr = wtr.reshape(H, 96, W, KH, OC)
    btr = bias.astype(np.float32) * np.float32(WSCALE)  # [o, h, w] exact

    in_maps = []
    for i in range(NCORES):
        h0 = i * RPC
        xcore = xp[:, h0 : h0 + RPC + 2, :, :]
        wcore = wtr[h0 : h0 + RPC]
        # bsc[g*32+o, h*16+q] = 2^8 * bias[o, h0+h, q*4+g]
        bcore = btr[:, h0 : h0 + RPC]  # [o, h, w]
        bcore = bcore.reshape(OC, RPC, W // NQ, NQ)
        bcore = (
            np.ascontiguousarray(bcore.transpose(3, 0, 1, 2))  # [g, o, h, q]
            .reshape(4 * OC, RPC * (W // NQ))
            .astype(F8E3)
        )

        in_maps.append(
            {
                "xs": np.ascontiguousarray(
                    xcore[:, :, 1 : W + 1].reshape(32, RPC + 2, W * B)
                ),
                "wt": np.ascontiguousarray(
                    wcore.reshape(RPC, 96, W * KH * OC)
                ),
                "bsc": bcore,
            }
        )
    return in_maps


def _run(in_maps, trace=False, tmpdir=None):
    from concourse.bass_utils import run_bass_kernel_spmd

    if "nc" not in _cache:
        _cache["nc"] = _build_nc()
    return run_bass_kernel_spmd(
        _cache["nc"], in_maps, list(range(NCORES)), trace=trace, tmpdir=tmpdir
    )


def _assemble(results):
    out = np.empty((B, OC, H, W), dtype=np.float32)
    inv = np.float32(1.0 / WSCALE)
    for i in range(NCORES):
        # res: [h, g*32+o, q*16+b], w = q*4+g
        res = (
            results[i]["out"].astype(np.float32).reshape(RPC, NQ, OC, W // NQ, B)
            * inv
        )
        # -> out[b, o, h, q*4+g]
        out[:, :, i * RPC : (i + 1) * RPC, :] = res.transpose(
            4, 2, 0, 3, 1
        ).reshape(B, OC, RPC, W)
    return out


def kernel(x, weight, bias):
    x = np.asarray(x)
    weight = np.asarray(weight)
    bias = np.asarray(bias)
    in_maps = _prep_inputs(x, weight, bias)
    results = _run(in_maps).results
    return _assemble(results)
